# revision 19
# baseline (speedup 1.0000x reference)
"""Trainium2 Bass kernel for sliding-window GQA attention block.

Problem: B=1, S=2048, HID=2048, NH=16 q-heads, NKV=4 kv-heads, HD=128,
WINDOW=512, causal; rotary embedding on q/k; projections wq/wk/wv/wo.

Sharding (8 cores): tensor-parallel over the 4 KV-head groups (4 q-heads
per group) x sequence-parallel over 2 halves of 1024 queries. Each core
computes its group's q/k/v projections for its sequence span (+512-key
halo), banded sliding-window attention, and a partial output projection
(emitted transposed, [od, q]). Host sums the 4 group-partials per half.

All tensors are bf16 on the wire and as matmul operands (PSUM accum is
f32). x is loaded once and stays resident in SBUF. One PSUM pool with
three 2-bank tag slots (k, v) + one double-buffered 4-bank slot (w)
serves every phase, so there are no pool-teardown stalls. Score matmuls
are interleaved at fine grain with q-projection / PV matmuls so the
Activation engine's exp() throughput (the phase-C co-bottleneck) hides
under PE work. Masking is geometric: 0/1 boundary tiles applied to
exp(scores) on GpSimd, plus zeroed rows in the all-ones denominator
matmul for the r=0 halo padding.
"""
import sys
import os

sys.path.insert(0, "/opt/trn_rl_repo")

import numpy as np
import ml_dtypes

import concourse.bass as bass
import concourse.mybir as mybir
from concourse import bacc
import concourse.tile as tile
from concourse.bass_utils import run_bass_kernel_spmd

BF16 = mybir.dt.bfloat16
F32 = mybir.dt.float32
NPBF16 = ml_dtypes.bfloat16

S, HID, NH, NKV, HD, WINDOW = 2048, 2048, 16, 4, 128, 512
NCORES = 8
SQ = 1024          # queries per core
SK = 1536          # keys per core (incl. 512 halo)
HT = HID // 128    # 16 hid tiles
NHC = NH // NKV    # 4 q-heads per core
TJ = SK // 128     # 12 key tiles
EXP = mybir.ActivationFunctionType.Exp


def _win(tj):
    """Query window [w0, w1) of key tile tj in core-local coordinates."""
    return max(0, 128 * tj - 512), min(SQ, 128 * tj + 128)


# score tiles grouped so one exp() instruction covers several key tiles;
# every matmul piece of <=512 stays inside one 2KB psum bank
GROUPS = [[4], [8, 9], [0, 11, 1, 10], [3, 2], [5], [6], [7]]
_P_OFF = [0] * TJ
_G_OFF = []   # pblock offset of each group
_G_W = []     # total width of each group
_off = 0
for _g in GROUPS:
    _G_OFF.append(_off)
    for _tj in _g:
        _w0, _w1 = _win(_tj)
        _P_OFF[_tj] = _off
        _off += _w1 - _w0
    _G_W.append(_off - _G_OFF[-1])
P_TOTAL = _off  # 5120


def _mix(a, b):
    """Interleave thunk lists a (primary) and b evenly, then run in order."""
    out = []
    if not b:
        out = list(a)
    elif not a:
        out = list(b)
    else:
        ratio = len(a) / len(b)
        ai = bi = 0
        while ai < len(a) or bi < len(b):
            if bi * ratio <= ai or ai >= len(a):
                if bi < len(b):
                    out.append(b[bi])
                    bi += 1
                else:
                    out.append(a[ai])
                    ai += 1
            else:
                out.append(a[ai])
                ai += 1
    for t in out:
        t()


def build_nc():
    nc = bacc.Bacc("TRN2", target_bir_lowering=False, debug=False)

    xt_d = nc.dram_tensor("xt", [HID, SK], BF16, kind="ExternalInput").ap()
    wqt_d = nc.dram_tensor("wqt", [HID, 512], BF16, kind="ExternalInput").ap()
    wkt_d = nc.dram_tensor("wkt", [128, HT * 128], BF16,
                           kind="ExternalInput").ap()
    wvt_d = nc.dram_tensor("wvt", [128, HT * 128], BF16,
                           kind="ExternalInput").ap()
    wot_d = nc.dram_tensor("wot", [512, HID], BF16, kind="ExternalInput").ap()
    csa_d = nc.dram_tensor("csa", [128, SK], BF16, kind="ExternalInput").ap()
    csb_d = nc.dram_tensor("csb", [128, SK], BF16, kind="ExternalInput").ap()
    comb_d = nc.dram_tensor("comb", [128, 256], BF16, kind="ExternalInput").ap()
    ident_d = nc.dram_tensor("ident", [128, 128], BF16,
                             kind="ExternalInput").ap()
    onesm_d = nc.dram_tensor("onesm", [128, SK], BF16,
                             kind="ExternalInput").ap()
    bnd_d = nc.dram_tensor("bnd", [128, 256], BF16, kind="ExternalInput").ap()
    out_d = nc.dram_tensor("out", [HID, SQ], BF16, kind="ExternalOutput").ap()

    with tile.TileContext(nc) as tc:
        with tc.tile_pool(name="persist", bufs=1) as pp, \
             tc.tile_pool(name="ps", bufs=1, space="PSUM") as ps:
            x_sb = pp.tile([128, HT * SK], BF16)       # 48KB/part, resident x
            wkt_sb = pp.tile([128, HT * 128], BF16)
            wvt_sb = pp.tile([128, HT * 128], BF16)
            wqt_sb = pp.tile([128, HT * 512], BF16)    # 16KB
            wot_sb = pp.tile([128, NHC * HID], BF16)   # 16KB
            csa_sb = pp.tile([128, SK], BF16)
            csb_sb = pp.tile([128, SK], BF16)
            comb_sb = pp.tile([128, 256], BF16)
            ident_sb = pp.tile([128, 128], BF16)
            onesm_sb = pp.tile([128, SK], BF16)
            bnd_sb = pp.tile([128, 256], BF16)
            kt_rot = pp.tile([128, SK], BF16)
            vt_sb = pp.tile([128, SK], BF16)           # v staging (d-major)
            vs_sb = pp.tile([128, SK], BF16)           # v s-major
            qt_rot = pp.tile([128, NHC * SQ], BF16)    # 8KB
            attnT = pp.tile([128, NHC * SQ], BF16)     # 8KB
            m1_sb = pp.tile([128, SQ], BF16)
            m2_sb = pp.tile([128, SQ], BF16)
            m1b_sb = pp.tile([128, SQ], BF16)
            m2b_sb = pp.tile([128, SQ], BF16)
            recip_sb = pp.tile([128, 512], F32)
            pblocks = [pp.tile([128, P_TOTAL], BF16, name=f"pblock{i}")
                       for i in range(4)]

            # ---- input DMAs across all three DMA paths (sync/scalar HWDGE
            # + gpsimd SWDGE); first tiles split for a faster cold start ----
            _dq = [nc.sync, nc.scalar, nc.gpsimd]

            nc.sync.dma_start(x_sb[:, 0:768], xt_d[0:128, 0:768])
            nc.scalar.dma_start(wkt_sb[:], wkt_d)
            nc.gpsimd.dma_start(wvt_sb[:], wvt_d)
            nc.sync.dma_start(x_sb[:, 768:SK], xt_d[0:128, 768:SK])
            nc.scalar.dma_start(x_sb[:, SK:SK + 768], xt_d[128:256, 0:768])
            nc.gpsimd.dma_start(x_sb[:, SK + 768:2 * SK],
                                xt_d[128:256, 768:SK])
            for ht in range(2, HT):
                _dq[ht % 3].dma_start(x_sb[:, SK * ht:SK * (ht + 1)],
                                      xt_d[128 * ht:128 * (ht + 1), :])
            nc.sync.dma_start(comb_sb[:], comb_d)
            nc.scalar.dma_start(ident_sb[:], ident_d)
            nc.gpsimd.dma_start(csa_sb[:], csa_d)
            nc.sync.dma_start(csb_sb[:], csb_d)
            for t in range(HT):
                _dq[t % 3].dma_start(
                    wqt_sb[:, 512 * t:512 * (t + 1)],
                    wqt_d[128 * t:128 * (t + 1), :])
            nc.sync.dma_start(onesm_sb[:], onesm_d)
            nc.scalar.dma_start(bnd_sb[:], bnd_d)
            for m in range(NHC):
                _dq[m % 3].dma_start(
                    wot_sb[:, HID * m:HID * (m + 1)],
                    wot_d[128 * m:128 * (m + 1), :])

            # ================= phase A: k/v (2 passes) + q0/q1 =================
            # pass-0 kv (768 keys) + full q0/q1 interleaved per hid-tile so
            # the PE pace (~1.5us/tile) matches the x DMA pace; pass-1 kv
            # runs afterwards from resident x with the q0/q1 ropes spliced in
            kp = [None, None]
            vp = [None, None]
            q_ps = [None] * NHC

            def kv_mm(p, which, ht):
                t = (kp if which == "k" else vp)[p]
                w_sb = wkt_sb if which == "k" else wvt_sb
                for lo, hi in ((0, 512), (512, 768)):
                    nc.tensor.matmul(
                        t[:, lo:hi],
                        w_sb[:, 128 * ht:128 * (ht + 1)],
                        x_sb[:, SK * ht + 768 * p + lo:SK * ht + 768 * p + hi],
                        start=(ht == 0), stop=(ht == HT - 1))

            def q_mm(h, ht):
                for sc in range(2):
                    nc.tensor.matmul(
                        q_ps[h][:, 512 * sc:512 * (sc + 1)],
                        wqt_sb[:, 512 * ht + 128 * h:512 * ht + 128 * (h + 1)],
                        x_sb[:, SK * ht + 512 + 512 * sc:
                             SK * ht + 512 + 512 * (sc + 1)],
                        start=(ht == 0), stop=(ht == HT - 1))

            def k_mul(p):
                sl = slice(768 * p, 768 * (p + 1))
                ma = m1_sb if p == 0 else m1b_sb
                mb = m2_sb if p == 0 else m2b_sb
                nc.vector.tensor_mul(ma[:, 0:768], kp[p][:], csa_sb[:, sl])
                nc.vector.tensor_mul(mb[:, 0:768], kp[p][:], csb_sb[:, sl])

            def k_comb(p):
                sl = slice(768 * p, 768 * (p + 1))
                ma = m1_sb if p == 0 else m1b_sb
                mb = m2_sb if p == 0 else m2b_sb
                for lo, hi in ((0, 512), (512, 768)):
                    nc.tensor.matmul(kp[p][:, lo:hi], comb_sb[:, 0:128],
                                     ma[:, lo:hi], start=True, stop=False)
                    nc.tensor.matmul(kp[p][:, lo:hi], comb_sb[:, 128:256],
                                     mb[:, lo:hi], start=False, stop=True)
                nc.scalar.copy(kt_rot[:, sl], kp[p][:])

            def v_stage(p):
                sl = slice(768 * p, 768 * (p + 1))
                nc.vector.tensor_copy(vt_sb[:, sl], vp[p][:])

            def q_mul(h):
                ma = m1_sb if h % 2 == 0 else m1b_sb
                mb = m2_sb if h % 2 == 0 else m2b_sb
                nc.vector.tensor_mul(ma[:], q_ps[h][:], csa_sb[:, 512:SK])
                nc.vector.tensor_mul(mb[:], q_ps[h][:], csb_sb[:, 512:SK])

            def q_comb(h):
                ma = m1_sb if h % 2 == 0 else m1b_sb
                mb = m2_sb if h % 2 == 0 else m2b_sb
                for sc in range(2):
                    sl = slice(512 * sc, 512 * (sc + 1))
                    nc.tensor.matmul(q_ps[h][:, sl], comb_sb[:, 0:128],
                                     ma[:, sl], start=True, stop=False)
                    nc.tensor.matmul(q_ps[h][:, sl], comb_sb[:, 128:256],
                                     mb[:, sl], start=False, stop=True)
                nc.vector.tensor_copy(qt_rot[:, SQ * h:SQ * (h + 1)],
                                      q_ps[h][:])

            def transpose_v():
                t_all = ps.tile([128, SK], BF16, tag="k", name="t_all")
                for tj in range(TJ):
                    sl = slice(128 * tj, 128 * (tj + 1))
                    nc.tensor.transpose(t_all[:, sl], vt_sb[:, sl],
                                        ident_sb[:])
                nc.vector.tensor_copy(vs_sb[:], t_all[:])

            # ================= phase C pieces =================
            _stag = [0]

            def scores_thunks(h):
                pblock = pblocks[h]
                thunks = []
                for gi, grp in enumerate(GROUPS):
                    def sc_t(gi=gi, grp=grp, h=h, pblock=pblock):
                        tag = "k" if _stag[0] % 2 == 0 else "v"
                        _stag[0] += 1
                        GW = _G_W[gi]
                        s_ps = ps.tile([128, 1024], F32, tag=tag,
                                       name="s_ps")
                        off = 0
                        for tj in grp:
                            w0, w1 = _win(tj)
                            W = w1 - w0
                            ktile = kt_rot[:, 128 * tj:128 * (tj + 1)]
                            qv = qt_rot[:, SQ * h + w0:SQ * h + w1]
                            pos = 0
                            while pos < W:
                                pw = min(512, W - pos)
                                nc.tensor.matmul(
                                    s_ps[:, off + pos:off + pos + pw],
                                    ktile, qv[:, pos:pos + pw],
                                    start=True, stop=True)
                                pos += pw
                            off += W
                        goff = _G_OFF[gi]
                        nc.scalar.activation(pblock[:, goff:goff + GW],
                                             s_ps[:, 0:GW], EXP)
                        for tj in grp:
                            w0, w1 = _win(tj)
                            W = w1 - w0
                            pt = pblock[:, _P_OFF[tj]:_P_OFF[tj] + W]
                            if tj >= 4:
                                nc.gpsimd.tensor_mul(pt[:, 0:128],
                                                     pt[:, 0:128],
                                                     bnd_sb[:, 0:128])
                            if tj <= 7:
                                nc.gpsimd.tensor_mul(pt[:, W - 128:W],
                                                     pt[:, W - 128:W],
                                                     bnd_sb[:, 128:256])
                    thunks.append(sc_t)
                return thunks

            def chunk_thunks(h, c, tag="w"):
                pblock = pblocks[h]
                od = [None]
                thunks = []
                order = [4 * c + 4] + [4 * c + k for k in
                                       (0, 1, 2, 3, 5, 6, 7)]
                for idx, tj in enumerate(order):
                    def ch_t(idx=idx, tj=tj, h=h, c=c, pblock=pblock):
                        if idx == 0:
                            od[0] = ps.tile([128, 1024], F32, tag=tag,
                                            name=f"od{h}_{c}")
                        w0, w1 = _win(tj)
                        W = w1 - w0
                        lo = max(0, 512 * c - w0)
                        hi = min(W, 512 * c + 512 - w0)
                        o_sl = slice(w0 + lo - 512 * c, w0 + hi - 512 * c)
                        d_sl = slice(512 + w0 + lo - 512 * c,
                                     512 + w0 + hi - 512 * c)
                        prhs = pblock[:, _P_OFF[tj] + lo:_P_OFF[tj] + hi]
                        st, sp = idx == 0, idx == len(order) - 1
                        nc.tensor.matmul(od[0][:, d_sl],
                                         onesm_sb[:, 128 * tj:128 * (tj + 1)],
                                         prhs, start=st, stop=sp,
                                         skip_group_check=True)
                        nc.tensor.matmul(od[0][:, o_sl],
                                         vs_sb[:, 128 * tj:128 * (tj + 1)],
                                         prhs, start=st, stop=sp,
                                         skip_group_check=True)
                    thunks.append(ch_t)

                def fin(h=h, c=c):
                    nc.vector.reciprocal_approx_fast(recip_sb[:],
                                                     od[0][:, 512:1024])
                    nc.vector.tensor_mul(
                        attnT[:, SQ * h + 512 * c:SQ * h + 512 * (c + 1)],
                        od[0][:, 0:512], recip_sb[:])
                thunks.append(fin)
                return thunks

            # ---- schedule ----
            # pass-0 kv + q0/q1, paced to the x DMA stream
            kp[0] = ps.tile([128, 768], F32, tag="k", name="k0")
            vp[0] = ps.tile([128, 768], F32, tag="v", name="v0")
            q_ps[0] = ps.tile([128, SQ], F32, tag="w", name="q_ps0")
            q_ps[1] = ps.tile([128, SQ], F32, tag="w", name="q_ps1")
            for ht in range(HT):
                kv_mm(0, "k", ht)
                kv_mm(0, "v", ht)
                q_mm(0, ht)
                q_mm(1, ht)
            # pass-1 v, with pass-0 k rope spliced in
            v_stage(0)
            k_mul(0)
            vp[1] = ps.tile([128, 768], F32, tag="v", name="v1")
            for ht in range(HT):
                kv_mm(1, "v", ht)
                if ht == 5:
                    k_comb(0)
            # pass-1 k, with q0/q1 ropes spliced in
            q_mul(0)
            kp[1] = ps.tile([128, 768], F32, tag="k", name="k1")
            for ht in range(HT):
                kv_mm(1, "k", ht)
                if ht == 5:
                    q_comb(0)
                    q_mul(1)
                elif ht == 11:
                    q_comb(1)
            k_mul(1)
            v_stage(1)

            # q2 + transposes + scores-h0 ; q3 + scores-h1
            q_ps[2] = ps.tile([128, SQ], F32, tag="w", name="q_ps2")
            q2t = [lambda ht=ht: q_mm(2, ht) for ht in range(HT)]
            q2t[0]()
            q2t[1]()
            k_comb(1)
            q2t[2]()
            q2t[3]()
            transpose_v()
            _mix(q2t[4:], scores_thunks(0))
            q_mul(2)

            q_ps[3] = ps.tile([128, SQ], F32, tag="w", name="q_ps3")
            q3t = [lambda ht=ht: q_mm(3, ht) for ht in range(HT)]
            for t in q3t[0:4]:
                t()
            q_comb(2)
            _mix(q3t[4:], scores_thunks(1))
            q_mul(3)

            # chunks interleaved with remaining scores; q3's rope is spliced
            # into the first chunk block
            ch0 = chunk_thunks(0, 0) + chunk_thunks(0, 1)
            ch0.insert(4, lambda: q_comb(3))
            _mix(ch0, scores_thunks(2))
            _mix(chunk_thunks(1, 0) + chunk_thunks(1, 1), scores_thunks(3))
            # the last head's chunks use the freed score slots (k/v) so their
            # psum allocation never waits on a recip/normalize of head 2
            for c in range(2):
                for t in chunk_thunks(2, c):
                    t()
            for c, tg in ((0, "k"), (1, "v")):
                for t in chunk_thunks(3, c, tag=tg):
                    t()

            # ============ phase D: output projection (transposed) ============
            with tc.tile_pool(name="ost", bufs=4) as ostp:
                ftags = ["k", "v", "w", "w"]
                for ot in range(HID // 128):
                    f_ps = ps.tile([128, SQ], F32, tag=ftags[ot % 4],
                                   name=f"f_ps{ot}")
                    for m in range(NHC):
                        wslice = wot_sb[:, HID * m + 128 * ot:
                                        HID * m + 128 * (ot + 1)]
                        for sc in range(2):
                            nc.tensor.matmul(
                                f_ps[:, 512 * sc:512 * (sc + 1)],
                                wslice,
                                attnT[:, SQ * m + 512 * sc:
                                      SQ * m + 512 * (sc + 1)],
                                start=(m == 0), stop=(m == NHC - 1),
                                skip_group_check=True)
                    stage = ostp.tile([128, SQ], BF16, tag="st")
                    if ot == HID // 128 - 1:
                        # drain the last tile with both engines + both queues
                        nc.scalar.copy(stage[:, 0:512], f_ps[:, 0:512])
                        nc.vector.tensor_copy(stage[:, 512:SQ],
                                              f_ps[:, 512:SQ])
                        nc.sync.dma_start(
                            out_d[128 * ot:128 * (ot + 1), 0:512],
                            stage[:, 0:512])
                        nc.scalar.dma_start(
                            out_d[128 * ot:128 * (ot + 1), 512:SQ],
                            stage[:, 512:SQ])
                    else:
                        if ot % 2 == 0:
                            nc.scalar.copy(stage[:], f_ps[:])
                        else:
                            nc.vector.tensor_copy(stage[:], f_ps[:])
                        _dq[ot % 2].dma_start(
                            out_d[128 * ot:128 * (ot + 1), :], stage[:])

    nc.compile()
    return nc


def host_inputs(x, wq, wk, wv, wo, freqs_cos, freqs_sin):
    """Build the 8 per-core input dicts (all bf16)."""
    xT = np.ascontiguousarray(np.asarray(x, dtype=np.float32)[0].T)  # [hid, s]
    wq = np.asarray(wq, dtype=np.float32)
    wk = np.asarray(wk, dtype=np.float32)
    wv = np.asarray(wv, dtype=np.float32)
    wo = np.asarray(wo, dtype=np.float32)
    cosT = np.asarray(freqs_cos, dtype=np.float32).T  # [64, S]
    sinT = np.asarray(freqs_sin, dtype=np.float32).T

    comb = np.zeros((128, 256), dtype=np.float32)
    for p in range(64):
        comb[p, p] = 1.0        # C1: out[p] = m1[p] - m1[p+64]
        comb[64 + p, p] = -1.0
        comb[p, 128 + 64 + p] = 1.0   # C2: out[64+p] = m2[p] + m2[p+64]
        comb[64 + p, 128 + 64 + p] = 1.0
    ident = np.eye(128, dtype=np.float32)
    y = np.arange(128)[None, :]
    xg = np.arange(128)[:, None]
    bnd = np.concatenate([(y >= xg).astype(np.float32),
                          (y <= xg).astype(np.float32)], axis=1)  # [128, 256]

    def b16(a):
        return np.ascontiguousarray(a.astype(NPBF16))

    in_maps = []
    for core in range(NCORES):
        g, r = core // 2, core % 2
        lo = 1024 * r - 512
        xt = np.zeros((HID, SK), dtype=np.float32)
        if r == 0:
            xt[:, 512:] = xT[:, 0:1024]
        else:
            xt[:, :] = xT[:, 512:2048]
        pos = np.clip(np.arange(lo, lo + SK), 0, S - 1)
        csa = np.concatenate([cosT[:, pos], sinT[:, pos]], axis=0)
        csb = np.concatenate([sinT[:, pos], cosT[:, pos]], axis=0)
        onesm = np.zeros((128, SK), dtype=np.float32)
        for tj in range(TJ):
            real = np.ones(128, dtype=np.float32) if r == 1 else \
                (128 * tj + np.arange(128) >= 512).astype(np.float32)
            onesm[:, 128 * tj:128 * (tj + 1)] = real[:, None]
        wktT = wk[128 * g:128 * (g + 1), :].T / np.sqrt(HD)   # [2048, 128]
        wvtT = wv[128 * g:128 * (g + 1), :].T
        wkt_tiled = wktT.reshape(HT, 128, 128).transpose(1, 0, 2).reshape(
            128, HT * 128)
        wvt_tiled = wvtT.reshape(HT, 128, 128).transpose(1, 0, 2).reshape(
            128, HT * 128)
        in_maps.append({
            "xt": b16(xt),
            "wqt": b16(wq[512 * g:512 * (g + 1), :].T),
            "wkt": b16(wkt_tiled),
            "wvt": b16(wvt_tiled),
            "wot": b16(wo[:, 512 * g:512 * (g + 1)].T),
            "csa": b16(csa),
            "csb": b16(csb),
            "comb": b16(comb),
            "ident": b16(ident),
            "onesm": b16(onesm),
            "bnd": b16(bnd),
        })
    return in_maps


def reduce_outputs(results):
    out = np.zeros((S, HID), dtype=np.float32)
    for core, res in enumerate(results):
        r = core % 2
        out[1024 * r:1024 * (r + 1), :] += \
            np.asarray(res["out"], dtype=np.float32).T
    return out[None]


_NC = None
_IN_MAPS = None


def _numpy_fallback(x, wq, wk, wv, wo, attention_mask, freqs_cos, freqs_sin):
    """Exact (slow) path for non-causal attention_mask inputs."""
    xs = np.asarray(x, np.float32)[0]
    cos = np.asarray(freqs_cos, np.float32)
    sin = np.asarray(freqs_sin, np.float32)

    def rope(t):
        x1, x2 = t[..., :64], t[..., 64:]
        c, s = cos[:, None, :], sin[:, None, :]
        return np.concatenate([x1 * c - x2 * s, x1 * s + x2 * c], axis=-1)

    q = rope((xs @ np.asarray(wq, np.float32).T).reshape(S, NH, HD))
    k = rope((xs @ np.asarray(wk, np.float32).T).reshape(S, NKV, HD))
    v = (xs @ np.asarray(wv, np.float32).T).reshape(S, NKV, HD)
    k = np.repeat(k, NH // NKV, axis=1)
    v = np.repeat(v, NH // NKV, axis=1)
    i = np.arange(S)[:, None]
    j = np.arange(S)[None, :]
    wmask = (i - j) > WINDOW
    out = np.zeros((S, NH, HD), np.float32)
    am = np.asarray(attention_mask, np.float32)[0, 0]
    for h in range(NH):
        sc = (q[:, h] @ k[:, h].T) / np.sqrt(HD) + am
        sc = np.where(wmask, -np.inf, sc)
        sc -= sc.max(axis=1, keepdims=True)
        p = np.exp(sc)
        p /= p.sum(axis=1, keepdims=True)
        out[:, h] = p @ v[:, h]
    return (out.reshape(S, NH * HD) @ np.asarray(wo, np.float32).T)[None]


def _is_standard_causal(attention_mask):
    am = np.asarray(attention_mask)
    if am.shape != (1, 1, S, S):
        return False
    i = np.arange(S)[:, None]
    j = np.arange(S)[None, :]
    expect = np.where(j > i, np.float32(-1e9), np.float32(0.0))
    return np.array_equal(am[0, 0], expect)


def kernel(x, wq, wk, wv, wo, attention_mask, freqs_cos, freqs_sin,
           **extra):
    global _NC, _IN_MAPS
    if not _is_standard_causal(attention_mask):
        return _numpy_fallback(x, wq, wk, wv, wo, attention_mask,
                               freqs_cos, freqs_sin)
    in_maps = host_inputs(x, wq, wk, wv, wo, freqs_cos, freqs_sin)
    _IN_MAPS = in_maps
    if _NC is None:
        _NC = build_nc()
    res = run_bass_kernel_spmd(_NC, in_maps, core_ids=list(range(NCORES)))
    return reduce_outputs(res.results)


if __name__ == "__main__":
    nc = build_nc()
    print("kernel built OK")


# revision 20
# speedup vs baseline: 1.0803x; 1.0803x over previous
"""Trainium2 Bass kernel for sliding-window GQA attention block.

Problem: B=1, S=2048, HID=2048, NH=16 q-heads, NKV=4 kv-heads, HD=128,
WINDOW=512, causal; rotary embedding on q/k; projections wq/wk/wv/wo.

Sharding (8 cores): tensor-parallel over the 4 KV-head groups (4 q-heads
per group) x sequence-parallel over 2 halves of 1024 queries. Each core
computes its group's q/k/v projections for its sequence span (+512-key
halo), banded sliding-window attention, and a partial output projection
(emitted transposed, [od, q]). Host sums the 4 group-partials per half.

All tensors are bf16 on the wire and as matmul operands (PSUM accum is
f32). x is loaded once and stays resident in SBUF. One PSUM pool with
three 2-bank tag slots (k, v) + one double-buffered 4-bank slot (w)
serves every phase, so there are no pool-teardown stalls. Score matmuls
are interleaved at fine grain with q-projection / PV matmuls so the
Activation engine's exp() throughput (the phase-C co-bottleneck) hides
under PE work. Masking is geometric: 0/1 boundary tiles applied to
exp(scores) on GpSimd, plus zeroed rows in the all-ones denominator
matmul for the r=0 halo padding.
"""
import sys
import os

sys.path.insert(0, "/opt/trn_rl_repo")

import numpy as np
import ml_dtypes

import concourse.bass as bass
import concourse.mybir as mybir
from concourse import bacc
import concourse.tile as tile
from concourse.bass_utils import run_bass_kernel_spmd

BF16 = mybir.dt.bfloat16
F32 = mybir.dt.float32
NPBF16 = ml_dtypes.bfloat16

S, HID, NH, NKV, HD, WINDOW = 2048, 2048, 16, 4, 128, 512
NCORES = 8
SQ = 1024          # queries per core
SK = 1536          # keys per core (incl. 512 halo)
HT = HID // 128    # 16 hid tiles
NHC = NH // NKV    # 4 q-heads per core
TJ = SK // 128     # 12 key tiles
EXP = mybir.ActivationFunctionType.Exp


def _win(tj):
    """Query window [w0, w1) of key tile tj in core-local coordinates."""
    return max(0, 128 * tj - 512), min(SQ, 128 * tj + 128)


# score tiles grouped so one exp() instruction covers several key tiles;
# every matmul piece of <=512 stays inside one 2KB psum bank
GROUPS = [[4], [8, 9], [0, 11, 1, 10], [3, 2], [5], [6], [7]]
_P_OFF = [0] * TJ
_G_OFF = []   # pblock offset of each group
_G_W = []     # total width of each group
_off = 0
for _g in GROUPS:
    _G_OFF.append(_off)
    for _tj in _g:
        _w0, _w1 = _win(_tj)
        _P_OFF[_tj] = _off
        _off += _w1 - _w0
    _G_W.append(_off - _G_OFF[-1])
P_TOTAL = _off  # 5120


def _mix(a, b):
    """Interleave thunk lists a (primary) and b evenly, then run in order."""
    out = []
    if not b:
        out = list(a)
    elif not a:
        out = list(b)
    else:
        ratio = len(a) / len(b)
        ai = bi = 0
        while ai < len(a) or bi < len(b):
            if bi * ratio <= ai or ai >= len(a):
                if bi < len(b):
                    out.append(b[bi])
                    bi += 1
                else:
                    out.append(a[ai])
                    ai += 1
            else:
                out.append(a[ai])
                ai += 1
    for t in out:
        t()


def build_nc():
    nc = bacc.Bacc("TRN2", target_bir_lowering=False, debug=False)

    xt_d = nc.dram_tensor("xt", [HID, SK], BF16, kind="ExternalInput").ap()
    wqt_d = nc.dram_tensor("wqt", [HID, 512], BF16, kind="ExternalInput").ap()
    wkt_d = nc.dram_tensor("wkt", [128, HT * 128], BF16,
                           kind="ExternalInput").ap()
    wvt_d = nc.dram_tensor("wvt", [128, HT * 128], BF16,
                           kind="ExternalInput").ap()
    wot_d = nc.dram_tensor("wot", [512, HID], BF16, kind="ExternalInput").ap()
    csa_d = nc.dram_tensor("csa", [128, SK], BF16, kind="ExternalInput").ap()
    csb_d = nc.dram_tensor("csb", [128, SK], BF16, kind="ExternalInput").ap()
    comb_d = nc.dram_tensor("comb", [128, 256], BF16, kind="ExternalInput").ap()
    ident_d = nc.dram_tensor("ident", [128, 128], BF16,
                             kind="ExternalInput").ap()
    onesm_d = nc.dram_tensor("onesm", [128, SK], BF16,
                             kind="ExternalInput").ap()
    bnd_d = nc.dram_tensor("bnd", [128, 256], BF16, kind="ExternalInput").ap()
    out_d = nc.dram_tensor("out", [HID, SQ], BF16, kind="ExternalOutput").ap()

    with tile.TileContext(nc) as tc:
        with tc.tile_pool(name="persist", bufs=1) as pp, \
             tc.tile_pool(name="ps", bufs=1, space="PSUM") as ps:
            x_sb = pp.tile([128, HT * SK], BF16)       # 48KB/part, resident x
            wkt_sb = pp.tile([128, HT * 128], BF16)
            wvt_sb = pp.tile([128, HT * 128], BF16)
            wqt_sb = pp.tile([128, HT * 512], BF16)    # 16KB
            wot_sb = pp.tile([128, NHC * HID], BF16)   # 16KB
            csa_sb = pp.tile([128, SK], BF16)
            csb_sb = pp.tile([128, SK], BF16)
            comb_sb = pp.tile([128, 256], BF16)
            ident_sb = pp.tile([128, 128], BF16)
            onesm_sb = pp.tile([128, SK], BF16)
            bnd_sb = pp.tile([128, 256], BF16)
            kt_rot = pp.tile([128, SK], BF16)
            vt_sb = pp.tile([128, SK], BF16)           # v staging (d-major)
            vs_sb = pp.tile([128, SK], BF16)           # v s-major
            qt_rot = pp.tile([128, NHC * SQ], BF16)    # 8KB
            attnT = pp.tile([128, NHC * SQ], BF16)     # 8KB
            m1_sb = pp.tile([128, SQ], BF16)
            m2_sb = pp.tile([128, SQ], BF16)
            m1b_sb = pp.tile([128, SQ], BF16)
            m2b_sb = pp.tile([128, SQ], BF16)
            recip_sb = pp.tile([128, 512], F32)
            pblocks = [pp.tile([128, P_TOTAL], BF16, name=f"pblock{i}")
                       for i in range(4)]

            # ---- input DMAs across the two HWDGE queues; first tiles split
            # for a faster cold start ----
            _dq = [nc.sync, nc.scalar]

            nc.sync.dma_start(x_sb[:, 0:768], xt_d[0:128, 0:768])
            nc.scalar.dma_start(wkt_sb[:], wkt_d)
            nc.sync.dma_start(x_sb[:, 768:SK], xt_d[0:128, 768:SK])
            nc.scalar.dma_start(wvt_sb[:], wvt_d)
            nc.sync.dma_start(x_sb[:, SK:SK + 768], xt_d[128:256, 0:768])
            nc.scalar.dma_start(x_sb[:, SK + 768:2 * SK],
                                xt_d[128:256, 768:SK])
            for ht in range(2, HT):
                _dq[ht % 2].dma_start(x_sb[:, SK * ht:SK * (ht + 1)],
                                      xt_d[128 * ht:128 * (ht + 1), :])
            nc.sync.dma_start(comb_sb[:], comb_d)
            nc.scalar.dma_start(ident_sb[:], ident_d)
            nc.sync.dma_start(csa_sb[:], csa_d)
            nc.scalar.dma_start(csb_sb[:], csb_d)
            for t in range(HT):
                _dq[t % 2].dma_start(
                    wqt_sb[:, 512 * t:512 * (t + 1)],
                    wqt_d[128 * t:128 * (t + 1), :])
            nc.sync.dma_start(onesm_sb[:], onesm_d)
            nc.scalar.dma_start(bnd_sb[:], bnd_d)
            for m in range(NHC):
                _dq[m % 2].dma_start(
                    wot_sb[:, HID * m:HID * (m + 1)],
                    wot_d[128 * m:128 * (m + 1), :])

            # ================= phase A: k/v (2 passes) + q0/q1 =================
            # pass-0 kv (768 keys) + full q0/q1 interleaved per hid-tile so
            # the PE pace (~1.5us/tile) matches the x DMA pace; pass-1 kv
            # runs afterwards from resident x with the q0/q1 ropes spliced in
            kp = [None, None]
            vp = [None, None]
            q_ps = [None] * NHC

            def kv_mm(p, which, ht):
                t = (kp if which == "k" else vp)[p]
                w_sb = wkt_sb if which == "k" else wvt_sb
                for lo, hi in ((0, 512), (512, 768)):
                    nc.tensor.matmul(
                        t[:, lo:hi],
                        w_sb[:, 128 * ht:128 * (ht + 1)],
                        x_sb[:, SK * ht + 768 * p + lo:SK * ht + 768 * p + hi],
                        start=(ht == 0), stop=(ht == HT - 1))

            def q_mm(h, ht):
                for sc in range(2):
                    nc.tensor.matmul(
                        q_ps[h][:, 512 * sc:512 * (sc + 1)],
                        wqt_sb[:, 512 * ht + 128 * h:512 * ht + 128 * (h + 1)],
                        x_sb[:, SK * ht + 512 + 512 * sc:
                             SK * ht + 512 + 512 * (sc + 1)],
                        start=(ht == 0), stop=(ht == HT - 1))

            def k_mul(p):
                sl = slice(768 * p, 768 * (p + 1))
                ma = m1_sb if p == 0 else m1b_sb
                mb = m2_sb if p == 0 else m2b_sb
                nc.vector.tensor_mul(ma[:, 0:768], kp[p][:], csa_sb[:, sl])
                nc.vector.tensor_mul(mb[:, 0:768], kp[p][:], csb_sb[:, sl])

            def k_comb(p):
                sl = slice(768 * p, 768 * (p + 1))
                ma = m1_sb if p == 0 else m1b_sb
                mb = m2_sb if p == 0 else m2b_sb
                for lo, hi in ((0, 512), (512, 768)):
                    nc.tensor.matmul(kp[p][:, lo:hi], comb_sb[:, 0:128],
                                     ma[:, lo:hi], start=True, stop=False)
                    nc.tensor.matmul(kp[p][:, lo:hi], comb_sb[:, 128:256],
                                     mb[:, lo:hi], start=False, stop=True)
                nc.scalar.copy(kt_rot[:, sl], kp[p][:])

            def v_stage(p):
                sl = slice(768 * p, 768 * (p + 1))
                nc.vector.tensor_copy(vt_sb[:, sl], vp[p][:])

            def q_mul(h):
                ma = m1_sb if h % 2 == 0 else m1b_sb
                mb = m2_sb if h % 2 == 0 else m2b_sb
                nc.vector.tensor_mul(ma[:], q_ps[h][:], csa_sb[:, 512:SK])
                nc.vector.tensor_mul(mb[:], q_ps[h][:], csb_sb[:, 512:SK])

            def q_comb(h):
                ma = m1_sb if h % 2 == 0 else m1b_sb
                mb = m2_sb if h % 2 == 0 else m2b_sb
                for sc in range(2):
                    sl = slice(512 * sc, 512 * (sc + 1))
                    nc.tensor.matmul(q_ps[h][:, sl], comb_sb[:, 0:128],
                                     ma[:, sl], start=True, stop=False)
                    nc.tensor.matmul(q_ps[h][:, sl], comb_sb[:, 128:256],
                                     mb[:, sl], start=False, stop=True)
                nc.vector.tensor_copy(qt_rot[:, SQ * h:SQ * (h + 1)],
                                      q_ps[h][:])

            def transpose_v():
                t_all = ps.tile([128, SK], BF16, tag="k", name="t_all")
                for tj in range(TJ):
                    sl = slice(128 * tj, 128 * (tj + 1))
                    nc.tensor.transpose(t_all[:, sl], vt_sb[:, sl],
                                        ident_sb[:])
                nc.vector.tensor_copy(vs_sb[:], t_all[:])

            # ================= phase C pieces =================
            _stag = [0]

            def scores_thunks(h):
                pblock = pblocks[h]
                thunks = []
                for gi, grp in enumerate(GROUPS):
                    def sc_t(gi=gi, grp=grp, h=h, pblock=pblock):
                        tag = "k" if _stag[0] % 2 == 0 else "v"
                        _stag[0] += 1
                        GW = _G_W[gi]
                        s_ps = ps.tile([128, 1024], F32, tag=tag,
                                       name="s_ps")
                        off = 0
                        for tj in grp:
                            w0, w1 = _win(tj)
                            W = w1 - w0
                            ktile = kt_rot[:, 128 * tj:128 * (tj + 1)]
                            qv = qt_rot[:, SQ * h + w0:SQ * h + w1]
                            pos = 0
                            while pos < W:
                                pw = min(512, W - pos)
                                nc.tensor.matmul(
                                    s_ps[:, off + pos:off + pos + pw],
                                    ktile, qv[:, pos:pos + pw],
                                    start=True, stop=True)
                                pos += pw
                            off += W
                        goff = _G_OFF[gi]
                        nc.scalar.activation(pblock[:, goff:goff + GW],
                                             s_ps[:, 0:GW], EXP)
                        for tj in grp:
                            w0, w1 = _win(tj)
                            W = w1 - w0
                            pt = pblock[:, _P_OFF[tj]:_P_OFF[tj] + W]
                            if tj >= 4:
                                nc.gpsimd.tensor_mul(pt[:, 0:128],
                                                     pt[:, 0:128],
                                                     bnd_sb[:, 0:128])
                            if tj <= 7:
                                nc.gpsimd.tensor_mul(pt[:, W - 128:W],
                                                     pt[:, W - 128:W],
                                                     bnd_sb[:, 128:256])
                    thunks.append(sc_t)
                return thunks

            def chunk_thunks(h, c, tag="w"):
                pblock = pblocks[h]
                od = [None]
                thunks = []
                order = [4 * c + 4] + [4 * c + k for k in
                                       (0, 1, 2, 3, 5, 6, 7)]
                for idx, tj in enumerate(order):
                    def ch_t(idx=idx, tj=tj, h=h, c=c, pblock=pblock):
                        if idx == 0:
                            od[0] = ps.tile([128, 1024], F32, tag=tag,
                                            name=f"od{h}_{c}")
                        w0, w1 = _win(tj)
                        W = w1 - w0
                        lo = max(0, 512 * c - w0)
                        hi = min(W, 512 * c + 512 - w0)
                        o_sl = slice(w0 + lo - 512 * c, w0 + hi - 512 * c)
                        d_sl = slice(512 + w0 + lo - 512 * c,
                                     512 + w0 + hi - 512 * c)
                        prhs = pblock[:, _P_OFF[tj] + lo:_P_OFF[tj] + hi]
                        st, sp = idx == 0, idx == len(order) - 1
                        nc.tensor.matmul(od[0][:, d_sl],
                                         onesm_sb[:, 128 * tj:128 * (tj + 1)],
                                         prhs, start=st, stop=sp,
                                         skip_group_check=True)
                        nc.tensor.matmul(od[0][:, o_sl],
                                         vs_sb[:, 128 * tj:128 * (tj + 1)],
                                         prhs, start=st, stop=sp,
                                         skip_group_check=True)
                    thunks.append(ch_t)

                def fin(h=h, c=c):
                    nc.vector.reciprocal_approx_fast(recip_sb[:],
                                                     od[0][:, 512:1024])
                    nc.vector.tensor_mul(
                        attnT[:, SQ * h + 512 * c:SQ * h + 512 * (c + 1)],
                        od[0][:, 0:512], recip_sb[:])
                thunks.append(fin)
                return thunks

            # ---- schedule ----
            # pass-0 kv + q0/q1, paced to the x DMA stream
            kp[0] = ps.tile([128, 768], F32, tag="k", name="k0")
            vp[0] = ps.tile([128, 768], F32, tag="v", name="v0")
            q_ps[0] = ps.tile([128, SQ], F32, tag="w", name="q_ps0")
            q_ps[1] = ps.tile([128, SQ], F32, tag="w", name="q_ps1")
            for ht in range(HT):
                kv_mm(0, "k", ht)
                kv_mm(0, "v", ht)
                q_mm(0, ht)
                q_mm(1, ht)
            # pass-1 v, with pass-0 k rope spliced in
            v_stage(0)
            k_mul(0)
            vp[1] = ps.tile([128, 768], F32, tag="v", name="v1")
            for ht in range(HT):
                kv_mm(1, "v", ht)
                if ht == 5:
                    k_comb(0)
            # pass-1 k, with q0/q1 ropes spliced in
            q_mul(0)
            kp[1] = ps.tile([128, 768], F32, tag="k", name="k1")
            for ht in range(HT):
                kv_mm(1, "k", ht)
                if ht == 5:
                    q_comb(0)
                    q_mul(1)
                elif ht == 11:
                    q_comb(1)
            k_mul(1)
            v_stage(1)

            # q2 + transposes + scores-h0 ; q3 + scores-h1
            q_ps[2] = ps.tile([128, SQ], F32, tag="w", name="q_ps2")
            q2t = [lambda ht=ht: q_mm(2, ht) for ht in range(HT)]
            q2t[0]()
            q2t[1]()
            k_comb(1)
            q2t[2]()
            q2t[3]()
            transpose_v()
            _mix(q2t[4:], scores_thunks(0))
            q_mul(2)

            q_ps[3] = ps.tile([128, SQ], F32, tag="w", name="q_ps3")
            q3t = [lambda ht=ht: q_mm(3, ht) for ht in range(HT)]
            for t in q3t[0:4]:
                t()
            q_comb(2)
            _mix(q3t[4:], scores_thunks(1))
            q_mul(3)

            # chunks interleaved with remaining scores; q3's rope is spliced
            # into the first chunk block
            ch0 = chunk_thunks(0, 0) + chunk_thunks(0, 1)
            ch0.insert(4, lambda: q_comb(3))
            _mix(ch0, scores_thunks(2))
            _mix(chunk_thunks(1, 0) + chunk_thunks(1, 1), scores_thunks(3))
            # the last head's chunks use the freed score slots (k/v) so their
            # psum allocation never waits on a recip/normalize of head 2
            for c in range(2):
                for t in chunk_thunks(2, c):
                    t()
            for c, tg in ((0, "k"), (1, "v")):
                for t in chunk_thunks(3, c, tag=tg):
                    t()

            # ============ phase D: output projection (transposed) ============
            with tc.tile_pool(name="ost", bufs=4) as ostp:
                ftags = ["k", "v", "w", "w"]
                for ot in range(HID // 128):
                    f_ps = ps.tile([128, SQ], F32, tag=ftags[ot % 4],
                                   name=f"f_ps{ot}")
                    for m in range(NHC):
                        wslice = wot_sb[:, HID * m + 128 * ot:
                                        HID * m + 128 * (ot + 1)]
                        for sc in range(2):
                            nc.tensor.matmul(
                                f_ps[:, 512 * sc:512 * (sc + 1)],
                                wslice,
                                attnT[:, SQ * m + 512 * sc:
                                      SQ * m + 512 * (sc + 1)],
                                start=(m == 0), stop=(m == NHC - 1),
                                skip_group_check=True)
                    stage = ostp.tile([128, SQ], BF16, tag="st")
                    if ot == HID // 128 - 1:
                        # drain the last tile with both engines + both queues
                        nc.scalar.copy(stage[:, 0:512], f_ps[:, 0:512])
                        nc.vector.tensor_copy(stage[:, 512:SQ],
                                              f_ps[:, 512:SQ])
                        nc.sync.dma_start(
                            out_d[128 * ot:128 * (ot + 1), 0:512],
                            stage[:, 0:512])
                        nc.scalar.dma_start(
                            out_d[128 * ot:128 * (ot + 1), 512:SQ],
                            stage[:, 512:SQ])
                    else:
                        if ot % 2 == 0:
                            nc.scalar.copy(stage[:], f_ps[:])
                        else:
                            nc.vector.tensor_copy(stage[:], f_ps[:])
                        _dq[ot % 2].dma_start(
                            out_d[128 * ot:128 * (ot + 1), :], stage[:])

    nc.compile()
    return nc


def host_inputs(x, wq, wk, wv, wo, freqs_cos, freqs_sin):
    """Build the 8 per-core input dicts (all bf16)."""
    xT = np.ascontiguousarray(np.asarray(x, dtype=np.float32)[0].T)  # [hid, s]
    wq = np.asarray(wq, dtype=np.float32)
    wk = np.asarray(wk, dtype=np.float32)
    wv = np.asarray(wv, dtype=np.float32)
    wo = np.asarray(wo, dtype=np.float32)
    cosT = np.asarray(freqs_cos, dtype=np.float32).T  # [64, S]
    sinT = np.asarray(freqs_sin, dtype=np.float32).T

    comb = np.zeros((128, 256), dtype=np.float32)
    for p in range(64):
        comb[p, p] = 1.0        # C1: out[p] = m1[p] - m1[p+64]
        comb[64 + p, p] = -1.0
        comb[p, 128 + 64 + p] = 1.0   # C2: out[64+p] = m2[p] + m2[p+64]
        comb[64 + p, 128 + 64 + p] = 1.0
    ident = np.eye(128, dtype=np.float32)
    y = np.arange(128)[None, :]
    xg = np.arange(128)[:, None]
    bnd = np.concatenate([(y >= xg).astype(np.float32),
                          (y <= xg).astype(np.float32)], axis=1)  # [128, 256]

    def b16(a):
        return np.ascontiguousarray(a.astype(NPBF16))

    in_maps = []
    for core in range(NCORES):
        g, r = core // 2, core % 2
        lo = 1024 * r - 512
        xt = np.zeros((HID, SK), dtype=np.float32)
        if r == 0:
            xt[:, 512:] = xT[:, 0:1024]
        else:
            xt[:, :] = xT[:, 512:2048]
        pos = np.clip(np.arange(lo, lo + SK), 0, S - 1)
        csa = np.concatenate([cosT[:, pos], sinT[:, pos]], axis=0)
        csb = np.concatenate([sinT[:, pos], cosT[:, pos]], axis=0)
        onesm = np.zeros((128, SK), dtype=np.float32)
        for tj in range(TJ):
            real = np.ones(128, dtype=np.float32) if r == 1 else \
                (128 * tj + np.arange(128) >= 512).astype(np.float32)
            onesm[:, 128 * tj:128 * (tj + 1)] = real[:, None]
        wktT = wk[128 * g:128 * (g + 1), :].T / np.sqrt(HD)   # [2048, 128]
        wvtT = wv[128 * g:128 * (g + 1), :].T
        wkt_tiled = wktT.reshape(HT, 128, 128).transpose(1, 0, 2).reshape(
            128, HT * 128)
        wvt_tiled = wvtT.reshape(HT, 128, 128).transpose(1, 0, 2).reshape(
            128, HT * 128)
        in_maps.append({
            "xt": b16(xt),
            "wqt": b16(wq[512 * g:512 * (g + 1), :].T),
            "wkt": b16(wkt_tiled),
            "wvt": b16(wvt_tiled),
            "wot": b16(wo[:, 512 * g:512 * (g + 1)].T),
            "csa": b16(csa),
            "csb": b16(csb),
            "comb": b16(comb),
            "ident": b16(ident),
            "onesm": b16(onesm),
            "bnd": b16(bnd),
        })
    return in_maps


def reduce_outputs(results):
    out = np.zeros((S, HID), dtype=np.float32)
    for core, res in enumerate(results):
        r = core % 2
        out[1024 * r:1024 * (r + 1), :] += \
            np.asarray(res["out"], dtype=np.float32).T
    return out[None]


_NC = None
_IN_MAPS = None


def _numpy_fallback(x, wq, wk, wv, wo, attention_mask, freqs_cos, freqs_sin):
    """Exact (slow) path for non-causal attention_mask inputs."""
    xs = np.asarray(x, np.float32)[0]
    cos = np.asarray(freqs_cos, np.float32)
    sin = np.asarray(freqs_sin, np.float32)

    def rope(t):
        x1, x2 = t[..., :64], t[..., 64:]
        c, s = cos[:, None, :], sin[:, None, :]
        return np.concatenate([x1 * c - x2 * s, x1 * s + x2 * c], axis=-1)

    q = rope((xs @ np.asarray(wq, np.float32).T).reshape(S, NH, HD))
    k = rope((xs @ np.asarray(wk, np.float32).T).reshape(S, NKV, HD))
    v = (xs @ np.asarray(wv, np.float32).T).reshape(S, NKV, HD)
    k = np.repeat(k, NH // NKV, axis=1)
    v = np.repeat(v, NH // NKV, axis=1)
    i = np.arange(S)[:, None]
    j = np.arange(S)[None, :]
    wmask = (i - j) > WINDOW
    out = np.zeros((S, NH, HD), np.float32)
    am = np.asarray(attention_mask, np.float32)[0, 0]
    for h in range(NH):
        sc = (q[:, h] @ k[:, h].T) / np.sqrt(HD) + am
        sc = np.where(wmask, -np.inf, sc)
        sc -= sc.max(axis=1, keepdims=True)
        p = np.exp(sc)
        p /= p.sum(axis=1, keepdims=True)
        out[:, h] = p @ v[:, h]
    return (out.reshape(S, NH * HD) @ np.asarray(wo, np.float32).T)[None]


def _is_standard_causal(attention_mask):
    am = np.asarray(attention_mask)
    if am.shape != (1, 1, S, S):
        return False
    i = np.arange(S)[:, None]
    j = np.arange(S)[None, :]
    expect = np.where(j > i, np.float32(-1e9), np.float32(0.0))
    return np.array_equal(am[0, 0], expect)


def kernel(x, wq, wk, wv, wo, attention_mask, freqs_cos, freqs_sin,
           **extra):
    global _NC, _IN_MAPS
    if not _is_standard_causal(attention_mask):
        return _numpy_fallback(x, wq, wk, wv, wo, attention_mask,
                               freqs_cos, freqs_sin)
    in_maps = host_inputs(x, wq, wk, wv, wo, freqs_cos, freqs_sin)
    _IN_MAPS = in_maps
    if _NC is None:
        _NC = build_nc()
    res = run_bass_kernel_spmd(_NC, in_maps, core_ids=list(range(NCORES)))
    return reduce_outputs(res.results)


if __name__ == "__main__":
    nc = build_nc()
    print("kernel built OK")
